# revision 9
# baseline (speedup 1.0000x reference)
"""Trainium2 Bass kernel for nn_Decoder_14894946583396 (dense_mlp).

Reference computation:
    sized = broadcast(representation[B,1,R] -> [B,S,R])   (ones @ rep)
    h     = relu(sized @ W1^T + b1)                       [B,S,HID]
    out   = h @ W2^T + b2                                 [B,S,OUT]

Because every position s within batch b receives the identical input row
representation[b], the MLP output row is identical for all S positions:
    row[b] = relu(rep[b] @ W1^T + b1) @ W2^T + b2         [B,OUT]
    out[b, s, :] = row[b]  for all s

So the kernel computes the tiny per-batch MLP on the TensorEngine and then
broadcast-writes each row across S via wide SBUF->DRAM DMAs. Data-parallel
across 8 NeuronCores: 4 batches per core; weights replicated.

All device inputs (transposed weights, transposed activations, biases, a
ones row) are packed host-side into ONE [128, PKW] f32 tensor loaded by a
single DMA: every consumer then depends on one DMA semaphore, which keeps
every Matmult at <=1 sync wait (walrus rejects matmuls with 2+ waits).

Biases are folded into the PSUM accumulation as K=1 matmuls; the
row->128-partition broadcast is folded into layer 2 by replicating the
H^T column across the stationary operand's free dim with a 0-stride AP.
"""

import sys

import numpy as np

if "/opt/trn_rl_repo" not in sys.path:
    sys.path.insert(0, "/opt/trn_rl_repo")

B, S, R = 32, 1024, 1024
HID, OUT = 512, 1024
N_CORES = 8
BPC = B // N_CORES  # batches per core

RC = R // 128  # layer-1 contraction chunks
HC = HID // 128  # layer-2 contraction chunks
OC = OUT // 512  # 512-wide output column chunks

# packed input column offsets
W1OFF = 0  # [p, rc*HID + h]     = W1[h, rc*128+p]
W2OFF = W1OFF + RC * HID  # [p, hc*OUT + o]     = W2[o, hc*128+p]
XTOFF = W2OFF + HC * OUT  # [p, rc*BPC + m]     = rep[m, rc*128+p]
B1OFF = XTOFF + RC * BPC  # row 0: b1
B2OFF = B1OFF + HID  # row 0: b2
ONOFF = B2OFF + OUT  # row 0: 128 ones
PKW = ONOFF + 128

N_COPIES = 4  # row copies along the free dim of each broadcast tile
S_PER_DMA = 128 * N_COPIES  # s-positions covered per output DMA
N_DMAS = S // S_PER_DMA  # output DMAs per batch

_CACHED_NC = None


def _build_nc():
    import concourse.bass as bass
    import concourse.mybir as mybir
    from concourse.tile import TileContext, add_dep_helper

    f32 = mybir.dt.float32
    nc = bass.Bass()

    pk = nc.dram_tensor("pk", [128, PKW], f32, kind="ExternalInput")
    out = nc.dram_tensor("out", [BPC, S, OUT], f32, kind="ExternalOutput")

    with TileContext(nc) as tc:
        with (
            tc.tile_pool(name="const", bufs=1) as cpool,
            tc.tile_pool(name="psum_ht", bufs=2, space="PSUM") as pp_ht,
            tc.tile_pool(name="psum_bc", bufs=4, space="PSUM") as pp_bc,
        ):
            out_dmas = []
            pk_sb = cpool.tile([128, PKW], f32, tag="pk")
            nc.sync.dma_start(out=pk_sb[:, :], in_=pk[:, :])

            # ---- layer 1: HT[h, m] = relu(W1 @ X^T + b1), h on partitions ----
            ht_sb = cpool.tile([128, HC * BPC], f32, tag="ht")
            for hc in range(HC):
                ph = pp_ht.tile([128, BPC], f32, tag="ht")
                for rc in range(RC):
                    nc.tensor.matmul(
                        ph[:, :],
                        lhsT=pk_sb[
                            :,
                            W1OFF
                            + rc * HID
                            + hc * 128 : W1OFF
                            + rc * HID
                            + hc * 128
                            + 128,
                        ],
                        rhs=pk_sb[:, XTOFF + rc * BPC : XTOFF + (rc + 1) * BPC],
                        start=(rc == 0),
                        stop=False,
                    )
                # += b1 chunk (outer product with ones row)
                nc.tensor.matmul(
                    ph[:, :],
                    lhsT=pk_sb[0:1, B1OFF + hc * 128 : B1OFF + (hc + 1) * 128],
                    rhs=pk_sb[0:1, ONOFF : ONOFF + BPC],
                    start=False,
                    stop=True,
                )
                nc.scalar.activation(
                    ht_sb[:, hc * BPC : (hc + 1) * BPC],
                    ph[:, :],
                    mybir.ActivationFunctionType.Relu,
                )

            # ---- layer 2 + broadcast: tile[p, o] = Y[b, o] for all p ---------
            # lhsT = HT column b replicated along the free (M) dim (0-stride),
            # so every output partition computes the same row Y[b, :].
            for b in range(BPC):
                yt = cpool.tile([128, N_COPIES * OUT], f32, tag=f"yt{b}")
                # one writer engine per yt tile => output DMAs wait on 1 sem
                copy_eng = nc.vector if b % 2 == 0 else nc.scalar
                for oc in range(OC):
                    pb = pp_bc.tile([128, 512], f32, tag="bc")
                    for hc in range(HC):
                        col = hc * BPC + b
                        nc.tensor.matmul(
                            pb[:, :],
                            lhsT=ht_sb[:, col : col + 1].broadcast_to([128, 128]),
                            rhs=pk_sb[
                                :,
                                W2OFF
                                + hc * OUT
                                + oc * 512 : W2OFF
                                + hc * OUT
                                + oc * 512
                                + 512,
                            ],
                            start=(hc == 0),
                            stop=False,
                        )
                    # += b2 chunk (ones lhsT -> same bias on every partition)
                    last_mm = nc.tensor.matmul(
                        pb[:, :],
                        lhsT=pk_sb[0:1, ONOFF : ONOFF + 128],
                        rhs=pk_sb[0:1, B2OFF + oc * 512 : B2OFF + (oc + 1) * 512],
                        start=False,
                        stop=True,
                    )
                    for c in range(N_COPIES):
                        dst = yt[:, c * OUT + oc * 512 : c * OUT + oc * 512 + 512]
                        if copy_eng is nc.vector:
                            last_dve = nc.vector.tensor_copy(dst, pb[:, :])
                        else:
                            last_act = nc.scalar.activation(
                                dst, pb[:, :], mybir.ActivationFunctionType.Copy
                            )
                # each DMA writes S_PER_DMA consecutive s rows (all identical)
                for j in range(N_DMAS):
                    d = nc.sync.dma_start(
                        out=out[b, j * S_PER_DMA : (j + 1) * S_PER_DMA, :].rearrange(
                            "(p c) o -> p c o", c=N_COPIES
                        ),
                        in_=yt[:, :].rearrange("p (c o) -> p c o", o=OUT),
                    )
                    out_dmas.append(d)

            # The kernel-tail drain waits on every proc's final tick, but this
            # walrus allows at most ONE sync wait per instruction. Chain SP
            # nops, one dependency each, so SP's vector clock observes the
            # final tick of every DMA lane and engine before the drain —
            # add_sem_waits then elides all of them from the drain.
            tail = out_dmas[-8:] + [last_mm, last_act, last_dve]
            for d in tail:
                n = nc.sync.nop(nofuse=True)
                add_dep_helper(
                    n.ins, d.ins, sync=True, reason="observe final ticks pre-drain"
                )

    return nc


def _get_nc():
    global _CACHED_NC
    if _CACHED_NC is None:
        _CACHED_NC = _build_nc()
    return _CACHED_NC


def _prep_in_maps(representation, W1, b1, W2, b2):
    rep = np.asarray(representation, dtype=np.float32).reshape(B, R)
    w1 = np.asarray(W1, dtype=np.float32)
    w2 = np.asarray(W2, dtype=np.float32)
    b1 = np.asarray(b1, dtype=np.float32)
    b2 = np.asarray(b2, dtype=np.float32)

    base = np.zeros((128, PKW), dtype=np.float32)
    base[:, W1OFF : W1OFF + RC * HID] = (
        w1.T.reshape(RC, 128, HID).transpose(1, 0, 2).reshape(128, RC * HID)
    )
    base[:, W2OFF : W2OFF + HC * OUT] = (
        w2.T.reshape(HC, 128, OUT).transpose(1, 0, 2).reshape(128, HC * OUT)
    )
    base[0, B1OFF : B1OFF + HID] = b1
    base[0, B2OFF : B2OFF + OUT] = b2
    base[0, ONOFF : ONOFF + 128] = 1.0

    in_maps = []
    for c in range(N_CORES):
        xt = rep[c * BPC : (c + 1) * BPC].T  # [R, BPC]
        pkc = base.copy()
        pkc[:, XTOFF : XTOFF + RC * BPC] = (
            xt.reshape(RC, 128, BPC).transpose(1, 0, 2).reshape(128, RC * BPC)
        )
        in_maps.append({"pk": pkc})
    return in_maps


def run_sharded(representation, W1, b1, W2, b2, **run_kwargs):
    """Compile+run on 8 cores; returns (full_output, BassKernelResults)."""
    from concourse.bass_utils import run_bass_kernel_spmd

    nc = _get_nc()
    in_maps = _prep_in_maps(representation, W1, b1, W2, b2)
    res = run_bass_kernel_spmd(nc, in_maps, core_ids=list(range(N_CORES)), **run_kwargs)
    full = np.concatenate([r["out"] for r in res.results], axis=0)
    return full, res


def kernel(representation, size_matrix=None, W1=None, b1=None, W2=None, b2=None):
    # size_matrix only contributes its shape in the reference (ones_like);
    # its values are unused.
    full, _ = run_sharded(representation, W1, b1, W2, b2)
    return full


# revision 21
# speedup vs baseline: 1.0251x; 1.0251x over previous
"""Trainium2 Bass kernel for nn_Decoder_14894946583396 (dense_mlp).

Reference computation:
    sized = broadcast(representation[B,1,R] -> [B,S,R])   (ones @ rep)
    h     = relu(sized @ W1^T + b1)                       [B,S,HID]
    out   = h @ W2^T + b2                                 [B,S,OUT]

Because every position s within batch b receives the identical input row
representation[b], the MLP output row is identical for all S positions:
    row[b] = relu(rep[b] @ W1^T + b1) @ W2^T + b2         [B,OUT]
    out[b, s, :] = row[b]  for all s

The kernel computes the tiny per-batch MLP on the TensorEngine and
broadcast-writes each row across S with wide SBUF->DRAM DMAs.
Data-parallel across 8 NeuronCores: 4 batches per core, replicated
weights.

Device pipeline per core (fp32 throughout):
  1. two packed input DMAs: pk1 = {x^T, b1, ones, I4, W1^T},
     pk2 = {W2^T, b2}; packing keeps every consumer at one DMA semaphore
     (this walrus allows at most ONE sync wait per instruction).
  2. L1: H[m,h] = x @ W1^T via 8 accumulating matmuls with the tiny x^T
     chunk as the stationary operand (cheap LDWEIGHTS), bias folded in as
     a K=1 ones-matmul, relu on ScalarE.
  3. H -> H^T via 4 PE transposes (H^T needed as stationary for L2).
  4. L2: Y[m,o] = H @ W2^T + b2, 10 matmuls into 2 PSUM banks.
  5. Y rows moved to partition-0 tiles by tiny SBUF->SBUF DMAs (matmul
     operands must start at partition 0/32/64).
  6. Broadcast: K=1 matmul with a ones row as stationary -> [128,512]
     PSUM tiles where every partition holds row[b]; copied 4x along the
     free dim into [128, 4*OUT] SBUF tiles (one writer engine per tile).
  7. 8 output DMAs of 2 MiB each: out[b, 512 s-rows, :] <- tile.

A chain of single-dependency SP nops before the kernel tail makes SP's
vector clock observe every DMA lane and engine, so the final drain needs
no multi-semaphore wait (ISA limit: one sync wait per instruction).
"""

import sys

import numpy as np

if "/opt/trn_rl_repo" not in sys.path:
    sys.path.insert(0, "/opt/trn_rl_repo")

B, S, R = 32, 1024, 1024
HID, OUT = 512, 1024
N_CORES = 8
BPC = B // N_CORES  # batches per core

RC = R // 128  # layer-1 contraction chunks
HC = HID // 128  # layer-2 contraction chunks
OC = OUT // 512  # 512-wide output column chunks

# pk1 column offsets: [p, XTOFF + rc*BPC + m] = rep[m, rc*128+p], etc.
XTOFF = 0
B1OFF = XTOFF + RC * BPC  # row 0: b1
ONOFF = B1OFF + HID  # row 0: 128 ones
I4OFF = ONOFF + 128  # rows 0..3: 4x4 identity
W1OFF = I4OFF + BPC  # [p, W1OFF + rc*HID + h] = W1[h, rc*128+p]
PK1W = W1OFF + RC * HID

# pk2 column offsets
W2OFF = 0  # [p, W2OFF + hc*OUT + o] = W2[o, hc*128+p]
B2OFF = W2OFF + HC * OUT  # row 0: b2
PK2W = B2OFF + OUT

N_COPIES = 4  # row copies along the free dim of each broadcast tile
S_PER_DMA = 128 * N_COPIES  # s-positions covered per output DMA
N_DMAS = S // S_PER_DMA  # output DMAs per batch

_CACHED_NC = None


def _build_nc():
    import concourse.bass as bass
    import concourse.mybir as mybir
    from concourse.tile import TileContext, add_dep_helper

    f32 = mybir.dt.float32
    relu = mybir.ActivationFunctionType.Relu
    fcopy = mybir.ActivationFunctionType.Copy
    nc = bass.Bass()

    pk1 = nc.dram_tensor("pk1", [128, PK1W], f32, kind="ExternalInput")
    pk2 = nc.dram_tensor("pk2", [128, PK2W], f32, kind="ExternalInput")
    out = nc.dram_tensor("out", [BPC, S, OUT], f32, kind="ExternalOutput")

    with TileContext(nc) as tc:
        with (
            tc.tile_pool(name="const", bufs=1) as cpool,
            tc.tile_pool(name="psum_s", bufs=2, space="PSUM") as pp_s,
            tc.tile_pool(name="psum_t", bufs=2, space="PSUM") as pp_t,
            tc.tile_pool(name="psum_bc", bufs=4, space="PSUM") as pp_bc,
        ):
            # All input/intermediate DMAs ride SWDGE (gpsimd) lanes so the 8
            # HWDGE lanes stay fresh for the 8 output DMAs — a lane-reusing
            # HWDGE trigger carries a structural lane wait that would exceed
            # the one-sync-wait-per-instruction ISA limit.
            p1 = cpool.tile([128, PK1W], f32, tag="pk1")
            dma_pk1 = nc.gpsimd.dma_start(out=p1[:, :], in_=pk1[:, :])
            p2 = cpool.tile([128, PK2W], f32, tag="pk2")
            dma_pk2 = nc.gpsimd.dma_start(out=p2[:, :], in_=pk2[:, :])

            # ---- L1: H[m, h] = x @ W1^T + b1, relu -------------------------
            ph = pp_s.tile([BPC, HID], f32, tag="s")
            for rc in range(RC):
                nc.tensor.matmul(
                    ph[:, :],
                    lhsT=p1[:, XTOFF + rc * BPC : XTOFF + (rc + 1) * BPC],
                    rhs=p1[:, W1OFF + rc * HID : W1OFF + rc * HID + HID],
                    start=(rc == 0),
                    stop=False,
                )
            mm_bias1 = nc.tensor.matmul(
                ph[:, :],
                lhsT=p1[0:1, ONOFF : ONOFF + BPC],
                rhs=p1[0:1, B1OFF : B1OFF + HID],
                start=False,
                stop=True,
            )
            # pk2 arrives during L1; observing it here keeps later matmuls
            # at a single sync wait.
            add_dep_helper(mm_bias1.ins, dma_pk2.ins, sync=True, reason="observe pk2")
            h_sb = cpool.tile([BPC, HID], f32, tag="h")
            nc.scalar.activation(h_sb[:, :], ph[:, :], relu)

            # ---- H -> H^T (stationary operand for L2) ----------------------
            ht_sb = cpool.tile([128, HC * BPC], f32, tag="ht")
            for hc in range(HC):
                pt = pp_t.tile([128, BPC], f32, tag="t")
                nc.tensor.transpose(
                    pt[:, :],
                    h_sb[0:BPC, hc * 128 : (hc + 1) * 128],
                    p1[0:BPC, I4OFF : I4OFF + BPC],
                )
                nc.scalar.activation(
                    ht_sb[:, hc * BPC : (hc + 1) * BPC], pt[:, :], fcopy
                )

            # ---- L2: Y[m, o] = H @ W2^T + b2 -------------------------------
            y_sb = cpool.tile([BPC, OUT], f32, tag="y")
            for oc in range(OC):
                py = pp_s.tile([BPC, 512], f32, tag="s")
                for hc in range(HC):
                    nc.tensor.matmul(
                        py[:, :],
                        lhsT=ht_sb[:, hc * BPC : (hc + 1) * BPC],
                        rhs=p2[
                            :, W2OFF + hc * OUT + oc * 512 : W2OFF + hc * OUT + oc * 512 + 512
                        ],
                        start=(hc == 0),
                        stop=False,
                    )
                nc.tensor.matmul(
                    py[:, :],
                    lhsT=p1[0:1, ONOFF : ONOFF + BPC],
                    rhs=p2[0:1, B2OFF + oc * 512 : B2OFF + (oc + 1) * 512],
                    start=False,
                    stop=True,
                )
                nc.vector.tensor_copy(y_sb[:, oc * 512 : (oc + 1) * 512], py[:, :])

            # ---- move Y rows to partition 0 (matmul base-partition rule) ---
            yrows = []
            row_dmas = []
            for b in range(BPC):
                yr = cpool.tile([1, OUT], f32, tag=f"yr{b}")
                d = nc.gpsimd.dma_start(out=yr[0:1, :], in_=y_sb[b : b + 1, :])
                yrows.append(yr)
                row_dmas.append(d)

            # ---- broadcast rows across partitions, replicate, store --------
            out_dmas = []
            bc_idx = 0
            for b in range(BPC):
                yt = cpool.tile([128, N_COPIES * OUT], f32, tag=f"yt{b}")
                copy_eng = "dve" if b % 2 == 0 else "act"
                for oc in range(OC):
                    pb = pp_bc.tile([128, 512], f32, tag="bc")
                    mm = nc.tensor.matmul(
                        pb[:, :],
                        lhsT=p1[0:1, ONOFF : ONOFF + 128],
                        rhs=yrows[b][0:1, oc * 512 : (oc + 1) * 512],
                        start=True,
                        stop=True,
                    )
                    # Greedy lane observation: groups 1-3 have no natural
                    # lane wait, so each observes the next row-DMA's lane;
                    # later groups then only wait on their PSUM slot release.
                    if 1 <= bc_idx <= BPC - 1:
                        add_dep_helper(
                            mm.ins,
                            row_dmas[bc_idx].ins,
                            sync=True,
                            reason="observe next yrow lane",
                        )
                    bc_idx += 1
                    for c in range(N_COPIES):
                        dst = yt[:, c * OUT + oc * 512 : c * OUT + oc * 512 + 512]
                        if copy_eng == "dve":
                            last_copy = last_dve = nc.vector.tensor_copy(dst, pb[:, :])
                        else:
                            last_copy = last_act = nc.scalar.activation(
                                dst, pb[:, :], fcopy
                            )
                    last_mm = mm
                # each DMA writes S_PER_DMA consecutive s rows (all identical)
                for j in range(N_DMAS):
                    d = nc.sync.dma_start(
                        out=out[b, j * S_PER_DMA : (j + 1) * S_PER_DMA, :].rearrange(
                            "(p c) o -> p c o", c=N_COPIES
                        ),
                        in_=yt[:, :].rearrange("p (c o) -> p c o", o=OUT),
                    )
                    out_dmas.append(d)

            # The kernel-tail drain waits on every proc's final tick, but this
            # walrus allows at most ONE sync wait per instruction. Chain SP
            # nops, one dependency each, so SP's vector clock observes the
            # final tick of every DMA lane and engine before the drain.
            tail = out_dmas + [dma_pk1, dma_pk2] + row_dmas + [last_mm, last_act, last_dve]
            for d in tail:
                n = nc.sync.nop(nofuse=True)
                add_dep_helper(
                    n.ins, d.ins, sync=True, reason="observe final ticks pre-drain"
                )

    return nc


def _get_nc():
    global _CACHED_NC
    if _CACHED_NC is None:
        _CACHED_NC = _build_nc()
    return _CACHED_NC


def _prep_in_maps(representation, W1, b1, W2, b2):
    rep = np.asarray(representation, dtype=np.float32).reshape(B, R)
    w1 = np.asarray(W1, dtype=np.float32)
    w2 = np.asarray(W2, dtype=np.float32)
    b1 = np.asarray(b1, dtype=np.float32)
    b2 = np.asarray(b2, dtype=np.float32)

    base1 = np.zeros((128, PK1W), dtype=np.float32)
    base1[0, B1OFF : B1OFF + HID] = b1
    base1[0, ONOFF : ONOFF + 128] = 1.0
    base1[0:BPC, I4OFF : I4OFF + BPC] = np.eye(BPC, dtype=np.float32)
    base1[:, W1OFF : W1OFF + RC * HID] = (
        w1.T.reshape(RC, 128, HID).transpose(1, 0, 2).reshape(128, RC * HID)
    )

    pk2 = np.zeros((128, PK2W), dtype=np.float32)
    pk2[:, W2OFF : W2OFF + HC * OUT] = (
        w2.T.reshape(HC, 128, OUT).transpose(1, 0, 2).reshape(128, HC * OUT)
    )
    pk2[0, B2OFF : B2OFF + OUT] = b2

    in_maps = []
    for c in range(N_CORES):
        xt = rep[c * BPC : (c + 1) * BPC].T  # [R, BPC]
        pk1 = base1.copy()
        pk1[:, XTOFF : XTOFF + RC * BPC] = (
            xt.reshape(RC, 128, BPC).transpose(1, 0, 2).reshape(128, RC * BPC)
        )
        in_maps.append({"pk1": pk1, "pk2": pk2})
    return in_maps


def run_sharded(representation, W1, b1, W2, b2, **run_kwargs):
    """Compile+run on 8 cores; returns (full_output, BassKernelResults)."""
    from concourse.bass_utils import run_bass_kernel_spmd

    nc = _get_nc()
    in_maps = _prep_in_maps(representation, W1, b1, W2, b2)
    res = run_bass_kernel_spmd(nc, in_maps, core_ids=list(range(N_CORES)), **run_kwargs)
    full = np.concatenate([r["out"] for r in res.results], axis=0)
    return full, res


def kernel(representation, size_matrix=None, W1=None, b1=None, W2=None, b2=None):
    # size_matrix only contributes its shape in the reference (ones_like);
    # its values are unused.
    full, _ = run_sharded(representation, W1, b1, W2, b2)
    return full
